# revision 21
# baseline (speedup 1.0000x reference)
"""Trainium2 Bass kernel for CoordLSVotingWeighted (segment_reduce).

Strategy: data-parallel over batch B=8 across 8 NeuronCores (1 image/core).

Per image, on device (two w-chunks, 96+32 columns):
  - hard one-hot of argmax over 9 seg channels (matches softmax(seg*1e6))
  - unit-direction projection features via a custom fused DVE op:
      rinv = approx 1/(nx^2+ny^2)   (bitwise-NOT seed + 1 Newton step)
      t = softplus(w)*rinv ; u = t*nx ; m = u*ny ; R11 = u*nx
    softplus on the scalar engine (Exp then Ln, one ACT table set);
    R00 is never materialized: R00 = sp - R11, recovered on host from
    the sp-feature accumulators.
  - segment reduce on TensorE, G=4 w-columns packed per matmul:
      lhsT = interleaved L [w, {hot, hot*ch, hot*cw}, class]  (96 cols)
      rhs  = planar R [w-window, {sp, m, R11}, point]         (108 cols)
      PSUM [96, 108] accumulates; diagonal 24x27 blocks summed on host.
  - asymmetric w-chunks (96/32): the small tail chunk's feature chain
    finishes fast so the last matmul windows fire early
  - per-chunk input tensors spread over the sync and gpsimd DMA rings
    (each ring processes its dma_starts serially; rings run in parallel)
  - wide PE warmup matmuls keep the tensor engine's pstate up until the
    real accumulation windows are ready; the rhs window view iterates
    w innermost (stride 1) so the matmul streams at full rate
Host: assemble 2x2 systems in float64, pinv-solve, scale by HEIGHT.

Self-contained: only needs numpy / ml_dtypes / concourse (installed env).
"""

import os

import numpy as np

B = 8
H = 128
W = 128
NCLS = 9  # seg channels, class 0 = background
NPTS = 9
OC = 8
HEIGHT = 128.0
N_CORES = 8

G = 4            # w columns packed per matmul
NCH = 2          # w-chunks (asymmetric: the small tail chunk finishes fast)
WCS = [96, 32]   # w columns per chunk
W0S = [0, 96]    # chunk start columns

N_WARM = int(os.environ.get("KERNEL_WARM", "30"))
WARM_N = 512  # moving cols per warmup matmul

# 1-Newton reciprocal-approx constants (minimax over s in [1e-8, 1e8])
RC0 = -0.2355
RC1 = 2.0015

_cache: dict = {}


def _register_rinv():
    """Runtime-register the custom DVE op RINV_XY = recip1(x^2 + y^2)."""
    import concourse.dve_ops as dops
    from concourse.dve_spec import (
        Spec, Src0, Src1, C0, C1, AluOp, Bin, lower, _has_src1,
    )
    from concourse.dve_uop import DveOpSpec

    for o in dops.OPS:
        if o.name == "RINV_XY":
            return o

    s = Src0 * Src0 + Src1 * Src1
    nb = Bin(AluOp.BITWISE_NOT, s, s)
    y0 = nb * C0
    y1 = y0 * (C1 - s * y0)

    def _ref(in0, in1, s0, s1, imm2):
        ss = (in0.astype(np.float32) ** 2 + in1.astype(np.float32) ** 2).astype(
            np.float32
        )
        nbv = (~ss.view(np.int32)).view(np.float32)
        y0v = (nbv * np.float32(s0)).astype(np.float32)
        return (y0v * (np.float32(s1) - ss * y0v)).astype(np.float32)

    spec = Spec(body=y1, reference=_ref)
    opcode = dops._CUSTOM_DVE_ROW_BASE + len(dops.OPS)
    shas = {}
    for ver in ("v3", "v4"):
        try:
            shas[ver] = DveOpSpec(
                name="RINV_XY",
                opcode=opcode,
                uops=lower(spec, ver=ver),
                rd1_en=_has_src1(spec),
            ).sha(ver)
        except Exception:
            pass
    op = dops.DveOp("RINV_XY", spec, subdim=False, uops_sha=shas)
    dops.OPS.append(op)
    dops.CUSTOM_DVE_SPECS[op.name] = op.spec
    dops._SUB_OPCODE_FOR_NAME[op.name] = opcode
    return op


def _patch_act_tables():
    """Exp and Ln resolve only to natural_log_exp_and_others -> 1 table load."""
    import concourse.bacc as bacc
    import concourse.mybir as mybir

    A = mybir.ActivationFunctionType
    orig = bacc.get_activation_tables
    if getattr(orig, "_softplus_patched", False):
        return

    def patched(arch):
        out = {}
        for name, funcs in orig(arch).items():
            f = set(funcs)
            if name != "natural_log_exp_and_others":
                f.discard(A.Exp)
                f.discard(A.Ln)
            out[name] = f
        return out

    patched._softplus_patched = True
    bacc.get_activation_tables = patched


def _build_nc():
    import concourse.bacc as bacc
    import concourse.tile as tile
    import concourse.mybir as mybir
    from concourse.alu_op_type import AluOpType as Alu

    Act = mybir.ActivationFunctionType
    Axis = mybir.AxisListType
    f32 = mybir.dt.float32
    b16 = mybir.dt.bfloat16

    RINV = _register_rinv()
    _patch_act_tables()

    nc = bacc.Bacc(
        "TRN2", target_bir_lowering=False, debug=False, num_devices=N_CORES
    )
    w_d = [nc.dram_tensor(f"w{c}", [H, NPTS * WCS[c]], b16, kind="ExternalInput")
           for c in range(NCH)]
    nyx_d = [nc.dram_tensor(f"nyx{c}", [H, 2 * NPTS * WCS[c]], b16,
                            kind="ExternalInput")
             for c in range(NCH)]
    f16 = mybir.dt.float16
    seg_d = [nc.dram_tensor(f"seg{c}", [H, WCS[c] * NCLS], f16,
                            kind="ExternalInput")
             for c in range(NCH)]
    cwb_d = [nc.dram_tensor(f"cwb{c}", [H, WCS[c] * OC], b16,
                            kind="ExternalInput")
             for c in range(NCH)]
    chv_d = nc.dram_tensor("chv", [H, 2], b16, kind="ExternalInput")
    out_d = nc.dram_tensor("acc", [G * 24, G * 27], f32, kind="ExternalOutput")

    with tile.TileContext(nc) as tc:
        with (
            tc.tile_pool(name="main", bufs=1) as pool,
            tc.tile_pool(name="ps", bufs=1, space="PSUM") as psp,
        ):
            acc = psp.tile([G * 24, G * 27], f32, tag="acc")
            wmL = pool.tile([H, 2 * G * 24], b16, tag="wmL")
            wmR = pool.tile([H, WARM_N], b16, tag="wmR")
            wacc = psp.tile([G * 24, WARM_N], f32, tag="wacc")

            sgt = [pool.tile([H, WCS[c] * NCLS], f16, tag=f"sgt{c}",
                             name=f"sgt{c}")
                   for c in range(NCH)]
            nyxt = [pool.tile([H, 2 * NPTS * WCS[c]], b16, tag=f"nyxt{c}",
                              name=f"nyxt{c}")
                    for c in range(NCH)]
            wt = [pool.tile([H, NPTS * WCS[c]], b16, tag=f"wt{c}", name=f"wt{c}")
                  for c in range(NCH)]
            cwt = [pool.tile([H, WCS[c] * OC], b16, tag=f"cwt{c}", name=f"cwt{c}")
                   for c in range(NCH)]
            cht_t = pool.tile([H, 2], b16, tag="chtt")

            # three DMA rings (each processes its dma_starts serially, rings
            # run in parallel); chunk0 pieces lead on sync+gpsimd, scalar's
            # ring starts after its ACT table load and carries late pieces
            nc.sync.dma_start(out=nyxt[0][:, :], in_=nyx_d[0][:, :])
            nc.gpsimd.dma_start(out=sgt[0][:, :], in_=seg_d[0][:, :])
            nc.sync.dma_start(out=wt[0][:, :], in_=w_d[0][:, :])
            nc.gpsimd.dma_start(out=nyxt[1][:, :], in_=nyx_d[1][:, :])
            nc.sync.dma_start(out=cwt[0][:, :], in_=cwb_d[0][:, :])
            nc.gpsimd.dma_start(out=wt[1][:, :], in_=w_d[1][:, :])
            nc.sync.dma_start(out=cht_t[:, :], in_=chv_d[:, :])
            nc.sync.dma_start(out=sgt[1][:, :], in_=seg_d[1][:, :])
            nc.sync.dma_start(out=cwt[1][:, :], in_=cwb_d[1][:, :])

            # warmup matmuls keep PE pstate up while DMA + DVE run
            nc.vector.memset(wmL[:, :], 0)
            nc.vector.memset(wmR[:, :], 0)
            for i in range(N_WARM):
                nc.tensor.matmul(
                    wacc[:, :],
                    wmL[:, (i % 2) * G * 24 : (i % 2 + 1) * G * 24],
                    wmR[:, :],
                    start=True,
                    stop=True,
                )

            cht = cht_t[:, :].bitcast(f32)

            for c in range(NCH):
                WC = WCS[c]
                NFC = NPTS * WC
                sgf = sgt[c][:, :]
                nyv = nyxt[c][:, 0:NFC]
                nxv = nyxt[c][:, NFC : 2 * NFC]

                mxt = pool.tile([H, WC], f16, tag=f"mx{c}")
                ewt = pool.tile([H, NFC], b16, tag=f"ew{c}")
                rit = pool.tile([H, NFC], b16, tag=f"ri{c}")
                tt = pool.tile([H, NFC], b16, tag=f"tt{c}")
                ut = pool.tile([H, NFC], b16, tag=f"ut{c}")
                L = pool.tile([H, WC * 3 * OC], b16, tag=f"L{c}")
                R = pool.tile([H, 3 * NFC], b16, tag=f"R{c}")
                L4 = L[:, :].rearrange("q (w f k) -> q w f k", f=3, k=OC)

                # ---- scalar chain: softplus -> sp (R feature block 0)
                nc.scalar.activation(out=ewt[:, :], in_=wt[c][:, :], func=Act.Exp)
                nc.scalar.activation(
                    out=R[:, 0:NFC], in_=ewt[:, :], func=Act.Ln, bias=1.0
                )  # NFC per-chunk

                # ---- vector chain
                nc.vector._custom_dve(
                    RINV, out=rit[:, :], in0=nxv, in1=nyv, s0=RC0, s1=RC1
                )
                sg_wc = sgf.rearrange("q (w k) -> q w k", k=NCLS)
                nc.vector.tensor_reduce(
                    out=mxt[:, :], in_=sg_wc, axis=Axis.X, op=Alu.max
                )
                mx_b = mxt[:, :].unsqueeze(2).broadcast_to((H, WC, OC))
                hot = L4[:, :, 0, :]
                nc.vector.tensor_tensor(
                    out=hot, in0=sg_wc[:, :, 1:NCLS], in1=mx_b, op=Alu.is_equal
                )
                nc.vector.tensor_tensor(
                    out=tt[:, :], in0=R[:, 0:NFC], in1=rit[:, :], op=Alu.mult
                )
                nc.vector.tensor_tensor(
                    out=ut[:, :], in0=tt[:, :], in1=nxv, op=Alu.mult
                )
                u_b = ut[:, :].unsqueeze(1).broadcast_to((H, 2, NFC))
                nyx_r = nyxt[c][:, :].rearrange("q (b f) -> q b f", b=2)
                mr_out = R[:, NFC : 3 * NFC].rearrange("q (b f) -> q b f", b=2)
                nc.vector.tensor_tensor(out=mr_out, in0=u_b, in1=nyx_r, op=Alu.mult)
                cw_r = cwt[c][:, :].rearrange("q (w k) -> q w k", k=OC)
                nc.vector.tensor_tensor(
                    out=L4[:, :, 2, :], in0=hot, in1=cw_r, op=Alu.mult
                )
                # hotch on the scalar engine (per-partition scale)
                nc.scalar.mul(out=L4[:, :, 1, :], in_=hot, mul=cht)

                # ---- segment reduce on TensorE, G columns per matmul
                Rv = R[:, :].rearrange("q (f g w) -> q f g w", f=3, g=NPTS)
                nwin = WC // G
                for wi in range(nwin):
                    nc.tensor.matmul(
                        acc[:, :],
                        L[:, wi * G * 24 : (wi + 1) * G * 24],
                        Rv[:, :, :, wi * G : (wi + 1) * G],
                        start=(c == 0 and wi == 0),
                        stop=(c == NCH - 1 and wi == nwin - 1),
                    )

            outs = pool.tile([G * 24, G * 27], f32, tag="outs")
            nc.scalar.copy(out=outs[:, :], in_=acc[:, :])
            nc.scalar.dma_start(out=out_d[:, :], in_=outs[:, :])

    nc.compile()
    return nc


def _prep_inputs(seg, direct, w):
    """Host-side sharding/staging: dtype cast + layout permutation only."""
    import ml_dtypes

    bf16 = ml_dtypes.bfloat16
    seg4 = seg.reshape(B, H, W, NCLS)
    d5 = direct.reshape(B, H, W, NPTS, 2)
    w4 = w.reshape(B, H, W, NPTS)
    cw = ((np.arange(W, dtype=np.float32) + 0.5) / HEIGHT).astype(bf16)
    segs, nyxs, wbs, cwbs = [], [], [], []
    for c in range(NCH):
        w0, wc = W0S[c], WCS[c]
        segs.append(
            np.ascontiguousarray(
                seg4[:, :, w0 : w0 + wc, :].reshape(B, H, wc * NCLS)
            ).astype(np.float16)
        )
        dd = d5[:, :, w0 : w0 + wc]  # [B,H,wc,9,2]
        nyxs.append(
            np.ascontiguousarray(dd.transpose(0, 1, 4, 3, 2)[:, :, ::-1, :, :])
            .astype(bf16)
            .reshape(B, H, 2 * NPTS * wc)
        )
        wbs.append(
            np.ascontiguousarray(
                w4[:, :, w0 : w0 + wc, :].transpose(0, 1, 3, 2)
            )
            .astype(bf16)
            .reshape(B, H, NPTS * wc)
        )
        cwbc = np.ascontiguousarray(
            np.broadcast_to(cw[w0 : w0 + wc].reshape(1, wc, 1), (H, wc, OC))
        ).reshape(1, H, wc * OC)
        cwbs.append(np.broadcast_to(cwbc, (B, H, wc * OC)))
    chv = (
        ((np.arange(H, dtype=np.float32) + 0.5) / HEIGHT)
        .reshape(H, 1)
        .view(bf16)
        .reshape(1, H, 2)
    )
    chv = np.broadcast_to(chv, (B, H, 2))
    return segs, nyxs, wbs, cwbs, chv


def _solve_host(a96: np.ndarray) -> np.ndarray:
    """acc [96,108] fp32 -> p [OC, NPTS, 2] fp32 (float64 pinv like ref)."""
    a = a96.astype(np.float64)
    acc = np.zeros((24, 27), dtype=np.float64)
    cidx = (np.arange(27) // 9) * (NPTS * G) + (np.arange(27) % 9) * G
    for j in range(G):
        acc += a[j * 24 : (j + 1) * 24][:, cidx + j]
    H0, H1, H2 = acc[0:OC], acc[OC : 2 * OC], acc[2 * OC : 3 * OC]
    SP0, M0, D0 = H0[:, 0:9], H0[:, 9:18], H0[:, 18:27]
    SP1, M1, D1 = H1[:, 0:9], H1[:, 9:18], H1[:, 18:27]
    SP2, M2, D2 = H2[:, 0:9], H2[:, 9:18], H2[:, 18:27]
    A = SP0 - D0
    Bm = M0
    D = D0
    qx = (SP1 - D1) - M2
    qy = D2 - M1
    Rm = np.empty((OC, NPTS, 2, 2), dtype=np.float64)
    Rm[..., 0, 0] = A
    Rm[..., 0, 1] = -Bm
    Rm[..., 1, 0] = -Bm
    Rm[..., 1, 1] = D
    q = np.stack([qx, qy], axis=-1)
    Rp = np.linalg.pinv(Rm.reshape(-1, 2, 2)).reshape(Rm.shape)
    p = np.einsum("cpij,cpj->cpi", Rp, q) * HEIGHT
    return p.astype(np.float32)


def kernel(seg, direct, w):
    if "nc" not in _cache:
        _cache["nc"] = _build_nc()
    nc = _cache["nc"]

    seg = np.ascontiguousarray(np.asarray(seg, dtype=np.float32))
    direct = np.ascontiguousarray(np.asarray(direct, dtype=np.float32))
    w = np.ascontiguousarray(np.asarray(w, dtype=np.float32))
    segs, nyxs, wbs, cwbs, chv = _prep_inputs(seg, direct, w)

    in_maps = []
    for i in range(B):
        m = {"chv": chv[i]}
        for c in range(NCH):
            m[f"seg{c}"] = segs[c][i]
            m[f"nyx{c}"] = nyxs[c][i]
            m[f"w{c}"] = wbs[c][i]
            m[f"cwb{c}"] = np.ascontiguousarray(cwbs[c][i])
        in_maps.append(m)

    from concourse.bass_utils import run_bass_kernel_spmd

    trace = bool(int(os.environ.get("KERNEL_TRACE", "0")))
    res = run_bass_kernel_spmd(
        nc, in_maps, core_ids=list(range(N_CORES)), trace=trace
    )
    kernel._last_exec_ns = res.exec_time_ns
    kernel._last_results = res

    out = np.stack(
        [_solve_host(np.asarray(res.results[i]["acc"])) for i in range(B)], axis=0
    )
    return out


# revision 22
# speedup vs baseline: 1.0460x; 1.0460x over previous
"""Trainium2 Bass kernel for CoordLSVotingWeighted (segment_reduce).

Strategy: data-parallel over batch B=8 across 8 NeuronCores (1 image/core).

Per image, on device (two w-chunks, 96+32 columns):
  - hard one-hot of argmax over 9 seg channels (matches softmax(seg*1e6))
  - unit-direction projection features via a custom fused DVE op:
      rinv = approx 1/(nx^2+ny^2)   (bitwise-NOT seed + 1 Newton step)
      t = softplus(w)*rinv ; u = t*nx ; m = u*ny ; R11 = u*nx
    softplus on the scalar engine (Exp then Ln, one ACT table set);
    R00 is never materialized: R00 = sp - R11, recovered on host from
    the sp-feature accumulators.
  - segment reduce on TensorE, G=4 w-columns packed per matmul:
      lhsT = interleaved L [w, {hot, hot*ch, hot*cw}, class]  (96 cols)
      rhs  = planar R [w-window, {sp, m, R11}, point]         (108 cols)
      PSUM [96, 108] accumulates; diagonal 24x27 blocks summed on host.
  - asymmetric w-chunks (96/32): the small tail chunk's feature chain
    finishes fast so the last matmul windows fire early
  - per-chunk input tensors spread over the sync and gpsimd DMA rings
    (each ring processes its dma_starts serially; rings run in parallel)
  - wide PE warmup matmuls keep the tensor engine's pstate up until the
    real accumulation windows are ready; the rhs window view iterates
    w innermost (stride 1) so the matmul streams at full rate
Host: assemble 2x2 systems in float64, pinv-solve, scale by HEIGHT.

Self-contained: only needs numpy / ml_dtypes / concourse (installed env).
"""

import os

import numpy as np

B = 8
H = 128
W = 128
NCLS = 9  # seg channels, class 0 = background
NPTS = 9
OC = 8
HEIGHT = 128.0
N_CORES = 8

G = 4            # w columns packed per matmul
NCH = 2          # w-chunks (asymmetric: the small tail chunk finishes fast)
WCS = [96, 32]   # w columns per chunk
W0S = [0, 96]    # chunk start columns

N_WARM = int(os.environ.get("KERNEL_WARM", "30"))
WARM_N = 512  # moving cols per warmup matmul

# 1-Newton reciprocal-approx constants (minimax over s in [1e-8, 1e8])
RC0 = -0.2355
RC1 = 2.0015

_cache: dict = {}


def _register_rinv():
    """Runtime-register the custom DVE op RINV_XY = recip1(x^2 + y^2)."""
    import concourse.dve_ops as dops
    from concourse.dve_spec import (
        Spec, Src0, Src1, C0, C1, AluOp, Bin, lower, _has_src1,
    )
    from concourse.dve_uop import DveOpSpec

    for o in dops.OPS:
        if o.name == "RINV_XY":
            return o

    s = Src0 * Src0 + Src1 * Src1
    nb = Bin(AluOp.BITWISE_NOT, s, s)
    y0 = nb * C0
    y1 = y0 * (C1 - s * y0)

    def _ref(in0, in1, s0, s1, imm2):
        ss = (in0.astype(np.float32) ** 2 + in1.astype(np.float32) ** 2).astype(
            np.float32
        )
        nbv = (~ss.view(np.int32)).view(np.float32)
        y0v = (nbv * np.float32(s0)).astype(np.float32)
        return (y0v * (np.float32(s1) - ss * y0v)).astype(np.float32)

    spec = Spec(body=y1, reference=_ref)
    opcode = dops._CUSTOM_DVE_ROW_BASE + len(dops.OPS)
    shas = {}
    for ver in ("v3", "v4"):
        try:
            shas[ver] = DveOpSpec(
                name="RINV_XY",
                opcode=opcode,
                uops=lower(spec, ver=ver),
                rd1_en=_has_src1(spec),
            ).sha(ver)
        except Exception:
            pass
    op = dops.DveOp("RINV_XY", spec, subdim=False, uops_sha=shas)
    dops.OPS.append(op)
    dops.CUSTOM_DVE_SPECS[op.name] = op.spec
    dops._SUB_OPCODE_FOR_NAME[op.name] = opcode
    return op


def _patch_act_tables():
    """Exp and Ln resolve only to natural_log_exp_and_others -> 1 table load."""
    import concourse.bacc as bacc
    import concourse.mybir as mybir

    A = mybir.ActivationFunctionType
    orig = bacc.get_activation_tables
    if getattr(orig, "_softplus_patched", False):
        return

    def patched(arch):
        out = {}
        for name, funcs in orig(arch).items():
            f = set(funcs)
            if name != "natural_log_exp_and_others":
                f.discard(A.Exp)
                f.discard(A.Ln)
            out[name] = f
        return out

    patched._softplus_patched = True
    bacc.get_activation_tables = patched


def _build_nc():
    import concourse.bacc as bacc
    import concourse.tile as tile
    import concourse.mybir as mybir
    from concourse.alu_op_type import AluOpType as Alu

    Act = mybir.ActivationFunctionType
    Axis = mybir.AxisListType
    f32 = mybir.dt.float32
    b16 = mybir.dt.bfloat16

    RINV = _register_rinv()
    _patch_act_tables()

    nc = bacc.Bacc(
        "TRN2", target_bir_lowering=False, debug=False, num_devices=N_CORES
    )
    w_d = [nc.dram_tensor(f"w{c}", [H, NPTS * WCS[c]], b16, kind="ExternalInput")
           for c in range(NCH)]
    nyx_d = [nc.dram_tensor(f"nyx{c}", [H, 2 * NPTS * WCS[c]], b16,
                            kind="ExternalInput")
             for c in range(NCH)]
    seg_d = [nc.dram_tensor(f"seg{c}", [H, 2 * WCS[c] * NCLS], b16,
                            kind="ExternalInput")
             for c in range(NCH)]
    cwb_d = [nc.dram_tensor(f"cwb{c}", [H, WCS[c] * OC], b16,
                            kind="ExternalInput")
             for c in range(NCH)]
    chv_d = nc.dram_tensor("chv", [H, 2], b16, kind="ExternalInput")
    out_d = nc.dram_tensor("acc", [G * 24, G * 27], f32, kind="ExternalOutput")

    with tile.TileContext(nc) as tc:
        with (
            tc.tile_pool(name="main", bufs=1) as pool,
            tc.tile_pool(name="ps", bufs=1, space="PSUM") as psp,
        ):
            acc = psp.tile([G * 24, G * 27], f32, tag="acc")
            wmL = pool.tile([H, 2 * G * 24], b16, tag="wmL")
            wmR = pool.tile([H, WARM_N], b16, tag="wmR")
            wacc = psp.tile([G * 24, WARM_N], f32, tag="wacc")

            sgt = [pool.tile([H, 2 * WCS[c] * NCLS], b16, tag=f"sgt{c}",
                             name=f"sgt{c}")
                   for c in range(NCH)]
            nyxt = [pool.tile([H, 2 * NPTS * WCS[c]], b16, tag=f"nyxt{c}",
                              name=f"nyxt{c}")
                    for c in range(NCH)]
            wt = [pool.tile([H, NPTS * WCS[c]], b16, tag=f"wt{c}", name=f"wt{c}")
                  for c in range(NCH)]
            cwt = [pool.tile([H, WCS[c] * OC], b16, tag=f"cwt{c}", name=f"cwt{c}")
                   for c in range(NCH)]
            cht_t = pool.tile([H, 2], b16, tag="chtt")

            # three DMA rings (each processes its dma_starts serially, rings
            # run in parallel); chunk0 pieces lead on sync+gpsimd, scalar's
            # ring starts after its ACT table load and carries late pieces
            nc.sync.dma_start(out=nyxt[0][:, :], in_=nyx_d[0][:, :])
            nc.gpsimd.dma_start(out=sgt[0][:, :], in_=seg_d[0][:, :])
            nc.sync.dma_start(out=wt[0][:, :], in_=w_d[0][:, :])
            nc.gpsimd.dma_start(out=nyxt[1][:, :], in_=nyx_d[1][:, :])
            nc.sync.dma_start(out=cwt[0][:, :], in_=cwb_d[0][:, :])
            nc.gpsimd.dma_start(out=wt[1][:, :], in_=w_d[1][:, :])
            nc.sync.dma_start(out=cht_t[:, :], in_=chv_d[:, :])
            nc.sync.dma_start(out=sgt[1][:, :], in_=seg_d[1][:, :])
            nc.sync.dma_start(out=cwt[1][:, :], in_=cwb_d[1][:, :])

            # warmup matmuls keep PE pstate up while DMA + DVE run
            nc.vector.memset(wmL[:, :], 0)
            nc.vector.memset(wmR[:, :], 0)
            for i in range(N_WARM):
                nc.tensor.matmul(
                    wacc[:, :],
                    wmL[:, (i % 2) * G * 24 : (i % 2 + 1) * G * 24],
                    wmR[:, :],
                    start=True,
                    stop=True,
                )

            cht = cht_t[:, :].bitcast(f32)

            for c in range(NCH):
                WC = WCS[c]
                NFC = NPTS * WC
                sgf = sgt[c][:, :].bitcast(f32)
                nyv = nyxt[c][:, 0:NFC]
                nxv = nyxt[c][:, NFC : 2 * NFC]

                mxt = pool.tile([H, WC], f32, tag=f"mx{c}")
                ewt = pool.tile([H, NFC], b16, tag=f"ew{c}")
                rit = pool.tile([H, NFC], b16, tag=f"ri{c}")
                tt = pool.tile([H, NFC], b16, tag=f"tt{c}")
                ut = pool.tile([H, NFC], b16, tag=f"ut{c}")
                L = pool.tile([H, WC * 3 * OC], b16, tag=f"L{c}")
                R = pool.tile([H, 3 * NFC], b16, tag=f"R{c}")
                L4 = L[:, :].rearrange("q (w f k) -> q w f k", f=3, k=OC)

                # ---- scalar chain: softplus -> sp (R feature block 0)
                nc.scalar.activation(out=ewt[:, :], in_=wt[c][:, :], func=Act.Exp)
                nc.scalar.activation(
                    out=R[:, 0:NFC], in_=ewt[:, :], func=Act.Ln, bias=1.0
                )  # NFC per-chunk

                # ---- vector chain
                nc.vector._custom_dve(
                    RINV, out=rit[:, :], in0=nxv, in1=nyv, s0=RC0, s1=RC1
                )
                sg_wc = sgf.rearrange("q (w k) -> q w k", k=NCLS)
                nc.vector.tensor_reduce(
                    out=mxt[:, :], in_=sg_wc, axis=Axis.X, op=Alu.max
                )
                mx_b = mxt[:, :].unsqueeze(2).broadcast_to((H, WC, OC))
                hot = L4[:, :, 0, :]
                nc.vector.tensor_tensor(
                    out=hot, in0=sg_wc[:, :, 1:NCLS], in1=mx_b, op=Alu.is_equal
                )
                nc.vector.tensor_tensor(
                    out=tt[:, :], in0=R[:, 0:NFC], in1=rit[:, :], op=Alu.mult
                )
                nc.vector.tensor_tensor(
                    out=ut[:, :], in0=tt[:, :], in1=nxv, op=Alu.mult
                )
                u_b = ut[:, :].unsqueeze(1).broadcast_to((H, 2, NFC))
                nyx_r = nyxt[c][:, :].rearrange("q (b f) -> q b f", b=2)
                mr_out = R[:, NFC : 3 * NFC].rearrange("q (b f) -> q b f", b=2)
                nc.vector.tensor_tensor(out=mr_out, in0=u_b, in1=nyx_r, op=Alu.mult)
                cw_r = cwt[c][:, :].rearrange("q (w k) -> q w k", k=OC)
                nc.vector.tensor_tensor(
                    out=L4[:, :, 2, :], in0=hot, in1=cw_r, op=Alu.mult
                )
                # hotch on the scalar engine (per-partition scale)
                nc.scalar.mul(out=L4[:, :, 1, :], in_=hot, mul=cht)

                # ---- segment reduce on TensorE, G columns per matmul
                Rv = R[:, :].rearrange("q (f g w) -> q f g w", f=3, g=NPTS)
                nwin = WC // G
                for wi in range(nwin):
                    nc.tensor.matmul(
                        acc[:, :],
                        L[:, wi * G * 24 : (wi + 1) * G * 24],
                        Rv[:, :, :, wi * G : (wi + 1) * G],
                        start=(c == 0 and wi == 0),
                        stop=(c == NCH - 1 and wi == nwin - 1),
                    )

            outs = pool.tile([G * 24, G * 27], f32, tag="outs")
            nc.scalar.copy(out=outs[:, :], in_=acc[:, :])
            nc.scalar.dma_start(out=out_d[:, :], in_=outs[:, :])

    nc.compile()
    return nc


def _prep_inputs(seg, direct, w):
    """Host-side sharding/staging: dtype cast + layout permutation only."""
    import ml_dtypes

    bf16 = ml_dtypes.bfloat16
    seg4 = seg.reshape(B, H, W, NCLS)
    d5 = direct.reshape(B, H, W, NPTS, 2)
    w4 = w.reshape(B, H, W, NPTS)
    cw = ((np.arange(W, dtype=np.float32) + 0.5) / HEIGHT).astype(bf16)
    segs, nyxs, wbs, cwbs = [], [], [], []
    for c in range(NCH):
        w0, wc = W0S[c], WCS[c]
        segs.append(
            np.ascontiguousarray(
                seg4[:, :, w0 : w0 + wc, :].reshape(B, H, wc * NCLS)
            ).view(bf16)
        )
        dd = d5[:, :, w0 : w0 + wc]  # [B,H,wc,9,2]
        nyxs.append(
            np.ascontiguousarray(dd.transpose(0, 1, 4, 3, 2)[:, :, ::-1, :, :])
            .astype(bf16)
            .reshape(B, H, 2 * NPTS * wc)
        )
        wbs.append(
            np.ascontiguousarray(
                w4[:, :, w0 : w0 + wc, :].transpose(0, 1, 3, 2)
            )
            .astype(bf16)
            .reshape(B, H, NPTS * wc)
        )
        cwbc = np.ascontiguousarray(
            np.broadcast_to(cw[w0 : w0 + wc].reshape(1, wc, 1), (H, wc, OC))
        ).reshape(1, H, wc * OC)
        cwbs.append(np.broadcast_to(cwbc, (B, H, wc * OC)))
    chv = (
        ((np.arange(H, dtype=np.float32) + 0.5) / HEIGHT)
        .reshape(H, 1)
        .view(bf16)
        .reshape(1, H, 2)
    )
    chv = np.broadcast_to(chv, (B, H, 2))
    return segs, nyxs, wbs, cwbs, chv


def _solve_host(a96: np.ndarray) -> np.ndarray:
    """acc [96,108] fp32 -> p [OC, NPTS, 2] fp32 (float64 pinv like ref)."""
    a = a96.astype(np.float64)
    acc = np.zeros((24, 27), dtype=np.float64)
    cidx = (np.arange(27) // 9) * (NPTS * G) + (np.arange(27) % 9) * G
    for j in range(G):
        acc += a[j * 24 : (j + 1) * 24][:, cidx + j]
    H0, H1, H2 = acc[0:OC], acc[OC : 2 * OC], acc[2 * OC : 3 * OC]
    SP0, M0, D0 = H0[:, 0:9], H0[:, 9:18], H0[:, 18:27]
    SP1, M1, D1 = H1[:, 0:9], H1[:, 9:18], H1[:, 18:27]
    SP2, M2, D2 = H2[:, 0:9], H2[:, 9:18], H2[:, 18:27]
    A = SP0 - D0
    Bm = M0
    D = D0
    qx = (SP1 - D1) - M2
    qy = D2 - M1
    Rm = np.empty((OC, NPTS, 2, 2), dtype=np.float64)
    Rm[..., 0, 0] = A
    Rm[..., 0, 1] = -Bm
    Rm[..., 1, 0] = -Bm
    Rm[..., 1, 1] = D
    q = np.stack([qx, qy], axis=-1)
    Rp = np.linalg.pinv(Rm.reshape(-1, 2, 2)).reshape(Rm.shape)
    p = np.einsum("cpij,cpj->cpi", Rp, q) * HEIGHT
    return p.astype(np.float32)


def kernel(seg, direct, w):
    if "nc" not in _cache:
        _cache["nc"] = _build_nc()
    nc = _cache["nc"]

    seg = np.ascontiguousarray(np.asarray(seg, dtype=np.float32))
    direct = np.ascontiguousarray(np.asarray(direct, dtype=np.float32))
    w = np.ascontiguousarray(np.asarray(w, dtype=np.float32))
    segs, nyxs, wbs, cwbs, chv = _prep_inputs(seg, direct, w)

    in_maps = []
    for i in range(B):
        m = {"chv": chv[i]}
        for c in range(NCH):
            m[f"seg{c}"] = segs[c][i]
            m[f"nyx{c}"] = nyxs[c][i]
            m[f"w{c}"] = wbs[c][i]
            m[f"cwb{c}"] = np.ascontiguousarray(cwbs[c][i])
        in_maps.append(m)

    from concourse.bass_utils import run_bass_kernel_spmd

    trace = bool(int(os.environ.get("KERNEL_TRACE", "0")))
    res = run_bass_kernel_spmd(
        nc, in_maps, core_ids=list(range(N_CORES)), trace=trace
    )
    kernel._last_exec_ns = res.exec_time_ns
    kernel._last_results = res

    out = np.stack(
        [_solve_host(np.asarray(res.results[i]["acc"])) for i in range(B)], axis=0
    )
    return out
